# revision 21
# baseline (speedup 1.0000x reference)
"""Trainium2 Bass kernel for nn_Attention_6932077215914 (GQA attention layer).

Strategy (8 NeuronCores, tensor-parallel over heads + sequence-parallel dense):
  - Host prep: x -> x^T (bf16, hidden contraction dim on SBUF partitions,
    pre-tiled to [p, tchunk, hc, t'] so each 512-token slice group is ONE
    contiguous 16KB run per partition = one cheap DMA trigger), softmax scale
    folded into w_q, w_q/w_kv sharded by head/KV-group. bf16 compute, fp32
    PSUM accumulation.
  - Core c owns heads {2c, 2c+1} (KV group c//2). Within each core pair the
    KV projection is split: even cores compute K^T, odd cores V^T, and a
    per-batch 2-rank AllGather exchanges the halves (~45us end-to-end
    latency, so both batches' KV chains run FIRST).
  - Projection chain order: kv(b0), q0(b0), q1(b0), kv(b1) run as a prefix
    (so both exchanges are in flight early), then the remaining q chains of
    batch 1 are INTERLEAVED one chunk per attention q-group into batch 0's
    attention - the in-order PE always has independent matmul work while
    ScalarE exp / GpSimd select / DVE tree latencies resolve, which also
    keeps the PE's HAM clock-gate warm. All phase-1 chunk accumulators
    borrow slots of the attention score ring (PSUM: scores 3x2 banks +
    ctx 2 banks = 8).
  - Attention (per batch x local head x 512-q-group): scores TRANSPOSED,
    S^T[k, q] = K^T.T @ Q^T into [128, 1024] 2-bank PSUM tiles (k-tile
    pairs), causally trimmed, diagonal pairs emitted first, exp fused across
    off-diagonal pairs, 3-pair score/PV lookahead. Row sums: pairwise bf16
    tree adds split DVE (off-diagonal subtree) / GpSimd (diagonal subtree),
    then ONE ones[128,128] matmul folds partitions AND broadcasts the
    denominators (deferred by one q-group so PE never waits on the tree);
    reciprocal_approx_fast + one DVE multiply normalize ctx^T.
  - AllToAll (4 chunks: batch x local-head) redistributes ctx^T so each core
    owns a 256-token slice for the dense projection (gathered ctx^T
    stationary, w_dense moving, hl-major contraction order).
"""

import sys
import types

import numpy as np
import ml_dtypes

B, SQ, HIDDEN = 2, 2048, 2048
HEADS, GROUPS, KVC = 16, 4, 128
SCALE = KVC ** -0.5
NCORES = 8
T = B * SQ               # 4096 flattened tokens
TC = 512                 # t-chunk for QKV projection
NTC = T // TC            # 8
MASK_VAL = -1e30


def _install_ntff_hook():
    """boot() skips NTFF hook registration when the image's antenv lacks
    axon_hooks; recreate the tiny module so trace=True / BASS_TRACE works."""
    if "antenv.axon_hooks" in sys.modules:
        return
    try:
        from trn_agent_boot.trn_boot import _ntff_profile_via_ctypes
        hook = _ntff_profile_via_ctypes("/opt/axon/libaxon_pjrt.so")
    except Exception:
        return
    mod = types.ModuleType("antenv.axon_hooks")
    mod.get_axon_ntff_profile_hook = lambda: hook
    mod.set_axon_ntff_profile_hook = lambda h: None
    sys.modules["antenv.axon_hooks"] = mod


_install_ntff_hook()

_CACHE = {}


def _build():
    import concourse.bass as bass
    import concourse.mybir as mybir
    import concourse.tile as tile
    from concourse import bacc
    from concourse.bass import ts, ds

    BF16 = mybir.dt.bfloat16
    F32 = mybir.dt.float32
    AF = mybir.ActivationFunctionType

    nc = bacc.Bacc("TRN2", target_bir_lowering=False, debug=False,
                   num_devices=NCORES)

    xt = nc.dram_tensor("xt", [128, NTC, 16 * TC], BF16, kind="ExternalInput")
    wq = nc.dram_tensor("wq", [128, 16 * 256], BF16, kind="ExternalInput")
    # per-core HALF of the kv projection: even cores get w_k, odd cores w_v
    wkv = nc.dram_tensor("wkv", [128, 16 * 128], BF16, kind="ExternalInput")
    wd = nc.dram_tensor("wd", [128, 16 * HIDDEN], BF16, kind="ExternalInput")
    out = nc.dram_tensor("out", [512, HIDDEN], F32, kind="ExternalOutput")

    with tile.TileContext(nc) as tc:
        import contextlib
        with contextlib.ExitStack() as ctx:
            # ---- long-lived pools ----
            persist = ctx.enter_context(tc.tile_pool(name="persist", bufs=1))
            dram = ctx.enter_context(tc.tile_pool(name="dram", bufs=1, space="DRAM"))
            p1 = ctx.enter_context(tc.tile_pool(name="p1", bufs=2))
            p2 = p2s = None   # attention pools enter after xt_b0 frees

            ones128 = persist.tile([128, 128], BF16, name="ones128")
            nc.vector.memset(ones128[:], 1.0)

            wq_sb = persist.tile([128, 16, 256], BF16, name="wq_sb")
            nc.sync.dma_start(out=wq_sb[:], in_=wq.ap())
            wkv_sb = persist.tile([128, 16, 128], BF16, name="wkv_sb")
            nc.sync.dma_start(out=wkv_sb[:], in_=wkv.ap())

            q_res = [[persist.tile([128, SQ], BF16, name=f"q{h}{b}")
                      for b in range(B)] for h in range(2)]
            k_res = [persist.tile([128, SQ], BF16, name=f"k{b}") for b in range(B)]
            v_res = [persist.tile([128, 16, 128], BF16, name=f"v{b}") for b in range(B)]

            cc_in = [[dram.tile([NCORES, 128, 256], BF16, name=f"ccin{b}{h}")
                      for h in range(2)] for b in range(B)]
            cc_out = [[dram.tile([NCORES, 128, 256], BF16, name=f"ccout{b}{h}")
                       for h in range(2)] for b in range(B)]
            kv_in = [dram.tile([128, SQ], BF16, name=f"kvin{b}") for b in range(B)]
            kv_out = [dram.tile([2, 128, SQ], BF16, name=f"kvout{b}")
                      for b in range(B)]

            # ONE shared PSUM pool for the whole kernel: scores/chains/dense
            # ring 3x[128,1024] (6 banks) + ctx 2x[128,512] (2 banks)
            pps = ctx.enter_context(tc.tile_pool(name="pps", bufs=2,
                                                 space="PSUM"))

            # x^T for the prefix chains: batch-0 slices + batch-1 slices for
            # the kv chain; both freed after the prefix (batch-1 q chains
            # re-fetch their slices into the freed space)
            p1xA_cm = tc.tile_pool(name="p1xA", bufs=1)
            p1xA = p1xA_cm.__enter__()
            p1xB_cm = tc.tile_pool(name="p1xB", bufs=1)
            p1xB = p1xB_cm.__enter__()
            xt_b0 = p1xA.tile([128, 4, 16 * TC], BF16, name="xt_b0")
            xt_kv1 = p1xB.tile([128, 4, 16 * TC], BF16, name="xt_kv1")
            # slice 0 in halves so the first chain's hc0-7 matmuls start early
            nc.sync.dma_start(out=xt_b0[:, 0, 0:8 * TC],
                              in_=xt.ap()[:, 0, 0:8 * TC])
            nc.sync.dma_start(out=xt_b0[:, 0, 8 * TC:16 * TC],
                              in_=xt.ap()[:, 0, 8 * TC:16 * TC])
            for sc4 in range(1, NTC):
                dst = xt_b0 if sc4 < 4 else xt_kv1
                nc.sync.dma_start(out=dst[:, sc4 % 4, :],
                                  in_=xt.ap()[:, sc4, :])

            W_APS = {"kv": wkv_sb[:, :, :], "q0": wq_sb[:, :, 0:128],
                     "q1": wq_sb[:, :, 128:256]}
            DEST = {"q0": q_res[0], "q1": q_res[1]}
            kvt_cur = {}

            def emit_chain(kind, b, sc, xsb):
                """One 512-token projection chunk: 16 matmuls + drain copy.
                Borrows a score-ring PSUM slot (first 512 columns)."""
                ps = pps.tile([128, 1024], F32, tag="sc", bufs=3,
                              name="chps")
                for hc in range(16):
                    nc.tensor.matmul(ps[:, 0:512], W_APS[kind][:, hc, :],
                                     xsb[:, sc, ts(hc, TC)],
                                     start=(hc == 0), stop=(hc == 15))
                if kind != "kv":
                    nc.vector.tensor_copy(DEST[kind][b][:, ts(sc, TC)],
                                          ps[:, 0:512])
                    return
                if sc == 0:
                    kvt_cur[b] = p1.tile([128, 4 * TC], BF16, tag="kvt",
                                         name=f"kvt{b}")
                nc.vector.tensor_copy(kvt_cur[b][:, ts(sc, TC)], ps[:, 0:512])
                if sc == 3:
                    nc.sync.dma_start(out=kv_in[b][:], in_=kvt_cur[b][:])
                    # exchange K/V within the core pair (pair rank 0 = K)
                    nc.gpsimd.collective_compute(
                        "AllGather", mybir.AluOpType.bypass,
                        replica_groups=[[2 * i, 2 * i + 1]
                                        for i in range(NCORES // 2)],
                        ins=[kv_in[b].opt()],
                        outs=[kv_out[b].opt()])
                    nc.sync.dma_start(out=k_res[b][:], in_=kv_out[b][0, :, :])
                    for s4 in range(4):
                        nc.sync.dma_start(
                            out=v_res[b][:, ds(4 * s4, 4), :],
                            in_=kv_out[b][1, :, ts(s4, TC)],
                            transpose=True)

            # ---- attention machinery ----
            pend = [None]   # deferred (b, hl, qg, ctx_ps, padd16)

            def finish():
                # fold+broadcast denominators with ONE ones-matmul, then
                # normalize ctx^T and scatter it to the A2A bounce
                fb, fhl, fqg, ctx_ps, padd16 = pend[0]
                pend[0] = None
                bc = pps.tile([128, 1024], F32, tag="sc", bufs=3, name="bc")
                nc.tensor.matmul(bc[:, 0:512], ones128[:], padd16[:],
                                 start=True, stop=True)
                rinv = p2s.tile([128, 512], F32, tag="rinv", bufs=2)
                nc.vector.reciprocal_approx_fast(rinv[:], bc[:, 0:512])
                ctxt = p2.tile([128, 512], BF16, tag="ctxt", bufs=3)
                nc.vector.tensor_mul(ctxt[:], ctx_ps[:], rinv[:])
                # one trigger: rows (2qg+h)*128+p of cc_in <- ctxt[p, h*256+c]
                nc.sync.dma_start(
                    out=cc_in[fb][fhl][ds(2 * fqg, 2), :, :]
                        .rearrange("h p c -> p h c"),
                    in_=ctxt[:])
                if fqg == 3:
                    nc.gpsimd.collective_compute(
                        "AllToAll", mybir.AluOpType.bypass,
                        replica_groups=[list(range(NCORES))],
                        ins=[cc_in[fb][fhl].opt()],
                        outs=[cc_out[fb][fhl].opt()])

            def emit_attn_qg(b, hl, qg, filler=None):
                nkt = 4 * (qg + 1)    # causal 128-wide k-tiles
                npair = nkt // 2
                # E^T slab, flat [k-tile * 512 q] free layout
                et = p2.tile([128, 16 * 512], BF16, tag="et", bufs=2,
                             name="et")
                ctx_ps = pps.tile([128, 512], F32, tag="ctx", bufs=2,
                                  name="ctx")

                def off(kt):
                    r = kt - 4 * qg
                    return 128 * r if r > 0 else 0

                # causally-zero prefixes of the diagonal k-tiles
                for kt in range(4 * qg + 1, nkt):
                    nc.gpsimd.memset(et[:, ds(512 * kt, off(kt))], 0.0)

                def emit_scores(j):
                    # pair j: k-tiles 2j, 2j+1 -> one 2-bank tile
                    sc_ps = pps.tile([128, 1024], F32, tag="sc", bufs=3,
                                     name="sc")
                    for u in range(2):
                        kt = 2 * j + u
                        o = off(kt)
                        nc.tensor.matmul(
                            sc_ps[:, ds(512 * u + o, 512 - o)],
                            k_res[b][:, ts(kt, 128)],
                            q_res[hl][b][:, ds(qg * 512 + o, 512 - o)],
                            start=True, stop=True)
                    if 2 * j >= 4 * qg:
                        # diagonal pair: per-tile exp on the valid q-suffix +
                        # zero the 128-wide triangle
                        for u in range(2):
                            kt = 2 * j + u
                            o = off(kt)
                            nc.scalar.activation(
                                et[:, ds(512 * kt + o, 512 - o)],
                                sc_ps[:, ds(512 * u + o, 512 - o)], AF.Exp)
                            nc.gpsimd.affine_select(
                                out=et[:, ds(512 * kt + o, 128)],
                                in_=et[:, ds(512 * kt + o, 128)],
                                compare_op=mybir.AluOpType.is_ge,
                                fill=0.0, base=0, pattern=[[1, 128]],
                                channel_multiplier=-1)
                    else:
                        # off-diagonal pair: one fused exp
                        nc.scalar.activation(et[:, ds(512 * 2 * j, 1024)],
                                             sc_ps[:, :], AF.Exp)

                def emit_pv(j, is_last):
                    for u in range(2):
                        kt = 2 * j + u
                        o = off(kt) if kt > 0 else 0
                        nc.tensor.matmul(
                            ctx_ps[:, ds(o, 512 - o)],
                            v_res[b][:, kt, :],
                            et[:, ds(512 * kt + o, 512 - o)],
                            start=(kt == 0), stop=(is_last and u == 1),
                            skip_group_check=True)

                # pair order: kt0's pair first (it opens the ctx
                # accumulation), then diagonal pairs (their exp+select chain
                # is longest), then the rest
                diag = [p_ for p_ in range(npair) if 2 * p_ >= 4 * qg]
                order = ([p_ for p_ in (0,) if p_ not in diag] + diag
                         + [p_ for p_ in range(1, npair) if p_ not in diag])
                LOOK = 3
                for idx, p_ in enumerate(order):
                    emit_scores(p_)
                    if idx == 0 and pend[0] is not None:
                        finish()
                    if idx >= LOOK:
                        emit_pv(order[idx - LOOK],
                                order[idx - LOOK] == order[-1])
                # the interleaved projection chunk runs while the last
                # exps/selects of this q-group resolve
                if filler is not None:
                    filler()
                for idx in range(max(len(order) - LOOK, 0), len(order)):
                    emit_pv(order[idx], order[idx] == order[-1])

                # row sums: pairwise bf16 tree, diagonal subtree on GpSimd
                def tree(lo, hi, depth, eng, tagp):
                    if hi - lo == 1:
                        return et[:, ds(512 * lo, 512)]
                    mid = (lo + hi) // 2
                    a = tree(lo, mid, depth + 1, eng, tagp)
                    b_ = tree(mid, hi, depth + 1, eng, tagp)
                    t = p2s.tile([128, 512], BF16, tag=f"{tagp}{depth}",
                                 bufs=2, name="tr")
                    eng.tensor_add(t[:], a[:], b_[:])
                    return t

                if qg == 0:
                    padd16 = tree(0, 4, 0, nc.vector, "tr")
                else:
                    a = tree(0, 4 * qg, 1, nc.vector, "tr")
                    b_ = tree(4 * qg, nkt, 1, nc.gpsimd, "gtr")
                    padd16 = p2s.tile([128, 512], BF16, tag="tr0", bufs=2)
                    nc.vector.tensor_add(padd16[:], a[:], b_[:])
                pend[0] = (b, hl, qg, ctx_ps, padd16)

            # ---- emission schedule ----
            # prefix: both KV chains early (each exchange has ~45us latency)
            for sc in range(4):
                emit_chain("kv", 0, sc, xt_b0)
            for sc in range(4):
                emit_chain("q0", 0, sc, xt_b0)
            for sc in range(4):
                emit_chain("q1", 0, sc, xt_b0)
            for sc in range(4):
                emit_chain("kv", 1, sc, xt_kv1)
            p1xB_cm.__exit__(None, None, None)
            p1xA_cm.__exit__(None, None, None)
            p2 = ctx.enter_context(tc.tile_pool(name="p2", bufs=2))
            p2s = ctx.enter_context(tc.tile_pool(name="p2s", bufs=4))
            # re-fetch batch-1 x^T slices for its q chains (DMA is idle here)
            p1xC_cm = tc.tile_pool(name="p1xC", bufs=1)
            p1xC = p1xC_cm.__enter__()
            xt_q1 = p1xC.tile([128, 4, 16 * TC], BF16, name="xt_q1")
            for sc4 in range(4):
                nc.sync.dma_start(out=xt_q1[:, sc4, :],
                                  in_=xt.ap()[:, 4 + sc4, :])

            # attention schedule: batch-1 q-projection chunks interleaved one
            # per q-group (each filler runs while that q-group's last
            # exp/selects resolve); first two q-groups fillerless so the
            # re-fetched slices have time to land. q0(b1) chunk i must land
            # before attn(b1,h0) q-group i, q1(b1) chunk i before (b1,h1) qg i.
            fill = [None,
                    ("q0", 1, 0), ("q0", 1, 1), ("q0", 1, 2), ("q0", 1, 3),
                    ("q1", 1, 0), ("q1", 1, 1), ("q1", 1, 2), ("q1", 1, 3),
                    None, None, None, None, None, None, None]
            fi = 0
            wd_sb, g_all = None, None
            for ab, ahl in ((0, 0), (0, 1), (1, 0), (1, 1)):
                for qg in range(4):
                    if fi == 9:
                        # batch-1 q chains done: free their x^T, start the
                        # dense weight load, allocate gather tiles
                        p1xC_cm.__exit__(None, None, None)
                        wdp = ctx.enter_context(tc.tile_pool(name="wdp",
                                                             bufs=1))
                        wd_sb = wdp.tile([128, 16, HIDDEN], BF16,
                                         name="wd_sb")
                        nc.sync.dma_start(out=wd_sb[:], in_=wd.ap())
                        p3g = ctx.enter_context(tc.tile_pool(name="p3g",
                                                             bufs=1))
                        g_all = [[p3g.tile([128, NCORES, 256], BF16,
                                           name=f"g{b}{h}")
                                  for h in range(2)] for b in range(B)]
                    args = fill[fi]
                    fi += 1
                    emit_attn_qg(ab, ahl, qg,
                                 filler=(None if args is None else
                                         (lambda a=args:
                                          emit_chain(*a, xt_q1))))
            finish()
            for b in range(B):
                for hl in range(2):
                    nc.sync.dma_start(
                        out=g_all[b][hl][:],
                        in_=cc_out[b][hl].rearrange("i p s -> p i s"))

            # ---- dense projection on my 256-token slice per batch ----
            # accumulators borrow two score-ring slots (2 banks each)
            with tc.tile_pool(name="p3", bufs=2) as p3:
                for b in range(B):
                    for u in range(2):
                        o_ps = [pps.tile([128, 1024], F32, tag="sc", bufs=3,
                                         name=f"ops{h_}") for h_ in range(2)]
                        o_sb = p3.tile([128, HIDDEN], F32, tag="osb")
                        # hl-major so the first half only needs g_all[b][0]
                        for ec in range(16):
                            hl, i = ec // 8, ec % 8
                            for oc in range(4):
                                nc.tensor.matmul(
                                    o_ps[oc // 2][:, ts(oc % 2, 512)],
                                    g_all[b][hl][:, i, ts(u, 128)],
                                    wd_sb[:, 2 * i + hl, ts(oc, 512)],
                                    start=(ec == 0), stop=(ec == 15),
                                    skip_group_check=True)
                        nc.scalar.copy(o_sb[:, ts(0, 1024)], o_ps[0][:])
                        nc.vector.tensor_copy(o_sb[:, ts(1, 1024)],
                                              o_ps[1][:])
                        nc.sync.dma_start(
                            out=out.ap()[ds(b * 256 + u * 128, 128), :],
                            in_=o_sb[:])

    nc.compile()
    return nc


def kernel(x, w_q, w_kv, w_dense):
    from concourse.bass_utils import run_bass_kernel_spmd

    bf16 = ml_dtypes.bfloat16
    x = np.asarray(x, dtype=np.float32)
    w_q = np.asarray(w_q, dtype=np.float32)
    w_kv = np.asarray(w_kv, dtype=np.float32)
    w_dense = np.asarray(w_dense, dtype=np.float32)

    # x^T pre-tiled to [p, tchunk, hc, t'] so one DMA trigger moves a
    # 512-token slice of every hidden chunk as one 16KB run per partition
    xt = np.ascontiguousarray(
        x.reshape(T, HIDDEN).T.reshape(16, 128, NTC, TC).transpose(1, 2, 0, 3)
        .reshape(128, NTC, 16 * TC)
    ).astype(bf16)
    wq_s = (w_q * SCALE).astype(bf16)          # fold softmax scale into Q proj
    wkv_b = w_kv.astype(bf16)
    wd_b = w_dense.astype(bf16)

    def pretile(w):
        # [2048, e] -> SBUF layout [p, hc*e]: row p, col hc*e_sz + e
        e_sz = w.shape[1]
        return np.ascontiguousarray(
            w.reshape(16, 128, e_sz).transpose(1, 0, 2).reshape(128, 16 * e_sz))

    wd_t = pretile(wd_b)
    in_maps = []
    for c in range(NCORES):
        g = c // 2
        if c % 2 == 0:
            wkv_c = wkv_b[:, 128 * g:128 * (g + 1)]                # K half
        else:
            wkv_c = wkv_b[:, 512 + 128 * g:512 + 128 * (g + 1)]    # V half
        in_maps.append({
            "xt": xt,
            "wq": pretile(wq_s[:, 256 * c:256 * (c + 1)]),
            "wkv": pretile(wkv_c),
            "wd": wd_t,
        })

    if "nc" not in _CACHE:
        _CACHE["nc"] = _build()
    nc = _CACHE["nc"]

    res = run_bass_kernel_spmd(nc, in_maps, core_ids=list(range(NCORES)))
    kernel.last_results = res
    kernel.last_exec_time_ns = res.exec_time_ns

    out_full = np.empty((T, HIDDEN), dtype=np.float32)
    for c in range(NCORES):
        r = res.results[c]["out"]              # [512, 2048]
        for b in range(B):
            out_full[b * SQ + 256 * c: b * SQ + 256 * (c + 1), :] = \
                r[b * 256:(b + 1) * 256, :]
    return out_full.reshape(B, SQ, HIDDEN)


# revision 25
# speedup vs baseline: 1.1123x; 1.1123x over previous
"""Trainium2 Bass kernel for nn_Attention_6932077215914 (GQA attention layer).

Strategy (8 NeuronCores, tensor-parallel over heads + sequence-parallel dense):
  - Host prep: x -> x^T (bf16, hidden contraction dim on SBUF partitions,
    pre-tiled to [p, tchunk, hc, t'] so each 512-token slice group is ONE
    contiguous 16KB run per partition = one cheap DMA trigger), softmax scale
    folded into w_q, w_q/w_kv sharded by head/KV-group. bf16 compute, fp32
    PSUM accumulation.
  - Core c owns heads {2c, 2c+1} (KV group c//2). Within each core pair the
    KV projection is split: even cores compute K^T, odd cores V^T, and a
    per-batch 2-rank AllGather exchanges the halves (~45us end-to-end
    latency, so both batches' KV chains run FIRST).
  - Projection chain order: kv(b0), q0(b0), q1(b0), kv(b1) run as a prefix
    (so both exchanges are in flight early), then the remaining q chains of
    batch 1 are INTERLEAVED one chunk per attention q-group into batch 0's
    attention - the in-order PE always has independent matmul work while
    ScalarE exp / GpSimd select / DVE tree latencies resolve, which also
    keeps the PE's HAM clock-gate warm. All phase-1 chunk accumulators
    borrow slots of the attention score ring (PSUM: scores 3x2 banks +
    ctx 2 banks = 8).
  - Attention (per batch x local head x 512-q-group): scores TRANSPOSED,
    S^T[k, q] = K^T.T @ Q^T into [128, 1024] 2-bank PSUM tiles (k-tile
    pairs), causally trimmed, diagonal pairs emitted first, exp fused across
    off-diagonal pairs, 3-pair score/PV lookahead. Row sums: pairwise bf16
    tree adds split DVE (off-diagonal subtree) / GpSimd (diagonal subtree),
    then ONE ones[128,128] matmul folds partitions AND broadcasts the
    denominators (deferred by one q-group so PE never waits on the tree);
    reciprocal_approx_fast + one DVE multiply normalize ctx^T.
  - AllToAll (4 chunks: batch x local-head) redistributes ctx^T so each core
    owns a 256-token slice for the dense projection (gathered ctx^T
    stationary, w_dense moving, hl-major contraction order).
"""

import sys
import types

import numpy as np
import ml_dtypes

B, SQ, HIDDEN = 2, 2048, 2048
HEADS, GROUPS, KVC = 16, 4, 128
SCALE = KVC ** -0.5
NCORES = 8
T = B * SQ               # 4096 flattened tokens
TC = 512                 # t-chunk for QKV projection
NTC = T // TC            # 8
MASK_VAL = -1e30


def _install_ntff_hook():
    """boot() skips NTFF hook registration when the image's antenv lacks
    axon_hooks; recreate the tiny module so trace=True / BASS_TRACE works."""
    if "antenv.axon_hooks" in sys.modules:
        return
    try:
        from trn_agent_boot.trn_boot import _ntff_profile_via_ctypes
        hook = _ntff_profile_via_ctypes("/opt/axon/libaxon_pjrt.so")
    except Exception:
        return
    mod = types.ModuleType("antenv.axon_hooks")
    mod.get_axon_ntff_profile_hook = lambda: hook
    mod.set_axon_ntff_profile_hook = lambda h: None
    sys.modules["antenv.axon_hooks"] = mod


_install_ntff_hook()

_CACHE = {}


def _build():
    import concourse.bass as bass
    import concourse.mybir as mybir
    import concourse.tile as tile
    from concourse import bacc
    from concourse.bass import ts, ds

    BF16 = mybir.dt.bfloat16
    F32 = mybir.dt.float32
    AF = mybir.ActivationFunctionType

    nc = bacc.Bacc("TRN2", target_bir_lowering=False, debug=False,
                   num_devices=NCORES)

    xt = nc.dram_tensor("xt", [128, NTC, 16 * TC], BF16, kind="ExternalInput")
    wq = nc.dram_tensor("wq", [128, 16 * 256], BF16, kind="ExternalInput")
    # per-core HALF of the kv projection: even cores get w_k, odd cores w_v
    wkv = nc.dram_tensor("wkv", [128, 16 * 128], BF16, kind="ExternalInput")
    wd = nc.dram_tensor("wd", [128, 16 * HIDDEN], BF16, kind="ExternalInput")
    out = nc.dram_tensor("out", [512, HIDDEN], F32, kind="ExternalOutput")

    with tile.TileContext(nc) as tc:
        import contextlib
        with contextlib.ExitStack() as ctx:
            # ---- long-lived pools ----
            persist = ctx.enter_context(tc.tile_pool(name="persist", bufs=1))
            dram = ctx.enter_context(tc.tile_pool(name="dram", bufs=1, space="DRAM"))
            p1 = ctx.enter_context(tc.tile_pool(name="p1", bufs=2))
            p2 = p2s = None   # attention pools enter after xt_b0 frees

            ones128 = persist.tile([128, 128], BF16, name="ones128")
            nc.vector.memset(ones128[:], 1.0)

            wq_sb = persist.tile([128, 16, 256], BF16, name="wq_sb")
            nc.sync.dma_start(out=wq_sb[:], in_=wq.ap())
            wkv_sb = persist.tile([128, 16, 128], BF16, name="wkv_sb")
            nc.sync.dma_start(out=wkv_sb[:], in_=wkv.ap())

            q_res = [[persist.tile([128, SQ], BF16, name=f"q{h}{b}")
                      for b in range(B)] for h in range(2)]
            k_res = [persist.tile([128, SQ], BF16, name=f"k{b}") for b in range(B)]
            v_res = [persist.tile([128, 16, 128], BF16, name=f"v{b}") for b in range(B)]

            cc_in = [[dram.tile([NCORES, 128, 256], BF16, name=f"ccin{b}{h}")
                      for h in range(2)] for b in range(B)]
            cc_out = [[dram.tile([NCORES, 128, 256], BF16, name=f"ccout{b}{h}")
                       for h in range(2)] for b in range(B)]
            kv_in = [dram.tile([128, SQ], BF16, name=f"kvin{b}") for b in range(B)]
            kv_out = [dram.tile([2, 128, SQ], BF16, name=f"kvout{b}")
                      for b in range(B)]

            # ONE shared PSUM pool for the whole kernel: scores/chains/dense
            # ring 3x[128,1024] (6 banks) + ctx 2x[128,512] (2 banks)
            pps = ctx.enter_context(tc.tile_pool(name="pps", bufs=2,
                                                 space="PSUM"))

            # x^T for the prefix chains: batch-0 slices + batch-1 slices for
            # the kv chain; both freed after the prefix (batch-1 q chains
            # re-fetch their slices into the freed space)
            p1xA_cm = tc.tile_pool(name="p1xA", bufs=1)
            p1xA = p1xA_cm.__enter__()
            p1xB_cm = tc.tile_pool(name="p1xB", bufs=1)
            p1xB = p1xB_cm.__enter__()
            xt_b0 = p1xA.tile([128, 4, 16 * TC], BF16, name="xt_b0")
            xt_kv1 = p1xB.tile([128, 4, 16 * TC], BF16, name="xt_kv1")
            # slice 0 in halves so the first chain's hc0-7 matmuls start early
            # (slices 4-7 are requested after kv(b0) is staged, so the
            # KV-exchange input never queues behind them)
            nc.sync.dma_start(out=xt_b0[:, 0, 0:8 * TC],
                              in_=xt.ap()[:, 0, 0:8 * TC])
            nc.sync.dma_start(out=xt_b0[:, 0, 8 * TC:16 * TC],
                              in_=xt.ap()[:, 0, 8 * TC:16 * TC])
            for sc4 in range(1, 4):
                nc.sync.dma_start(out=xt_b0[:, sc4, :],
                                  in_=xt.ap()[:, sc4, :])

            # warm-up collective: absorbs the ncfw init cost so the first
            # real exchange doesn't pay it
            cw_in = dram.tile([1, 128], BF16, name="cw_in")
            cw_out = dram.tile([2, 1, 128], BF16, name="cw_out")
            nc.gpsimd.collective_compute(
                "AllGather", mybir.AluOpType.bypass,
                replica_groups=[[2 * i, 2 * i + 1]
                                for i in range(NCORES // 2)],
                ins=[cw_in.opt()], outs=[cw_out.opt()])

            W_APS = {"kv": wkv_sb[:, :, :], "q0": wq_sb[:, :, 0:128],
                     "q1": wq_sb[:, :, 128:256]}
            DEST = {"q0": q_res[0], "q1": q_res[1]}
            kvt_cur = {}

            def emit_chain(kind, b, sc, xsb):
                """One 512-token projection chunk: 16 matmuls + drain copy.
                Borrows a score-ring PSUM slot (first 512 columns)."""
                ps = pps.tile([128, 1024], F32, tag="sc", bufs=3,
                              name="chps")
                for hc in range(16):
                    nc.tensor.matmul(ps[:, 0:512], W_APS[kind][:, hc, :],
                                     xsb[:, sc, ts(hc, TC)],
                                     start=(hc == 0), stop=(hc == 15))
                if kind != "kv":
                    nc.vector.tensor_copy(DEST[kind][b][:, ts(sc, TC)],
                                          ps[:, 0:512])
                    return
                if sc == 0:
                    kvt_cur[b] = p1.tile([128, 4 * TC], BF16, tag="kvt",
                                         bufs=1, name=f"kvt{b}")
                nc.vector.tensor_copy(kvt_cur[b][:, ts(sc, TC)], ps[:, 0:512])
                if sc == 3:
                    nc.sync.dma_start(out=kv_in[b][:], in_=kvt_cur[b][:])
                    # exchange K/V within the core pair (pair rank 0 = K)
                    nc.gpsimd.collective_compute(
                        "AllGather", mybir.AluOpType.bypass,
                        replica_groups=[[2 * i, 2 * i + 1]
                                        for i in range(NCORES // 2)],
                        ins=[kv_in[b].opt()],
                        outs=[kv_out[b].opt()])
                    nc.sync.dma_start(out=k_res[b][:], in_=kv_out[b][0, :, :])
                    for s4 in range(4):
                        nc.sync.dma_start(
                            out=v_res[b][:, ds(4 * s4, 4), :],
                            in_=kv_out[b][1, :, ts(s4, TC)],
                            transpose=True)

            # ---- attention machinery ----
            pend = [None]   # deferred (b, hl, qg, ctx_ps, padd16)

            def finish():
                # fold+broadcast denominators with ONE ones-matmul, then
                # normalize ctx^T and scatter it to the A2A bounce
                fb, fhl, fqg, ctx_ps, padd16 = pend[0]
                pend[0] = None
                bc = pps.tile([128, 1024], F32, tag="sc", bufs=3, name="bc")
                nc.tensor.matmul(bc[:, 0:512], ones128[:], padd16[:],
                                 start=True, stop=True)
                rinv = p2s.tile([128, 512], F32, tag="rinv", bufs=2)
                nc.vector.reciprocal_approx_fast(rinv[:], bc[:, 0:512])
                ctxt = p2.tile([128, 512], BF16, tag="ctxt", bufs=3)
                nc.vector.tensor_mul(ctxt[:], ctx_ps[:], rinv[:])
                # one trigger: rows (2qg+h)*128+p of cc_in <- ctxt[p, h*256+c]
                nc.sync.dma_start(
                    out=cc_in[fb][fhl][ds(2 * fqg, 2), :, :]
                        .rearrange("h p c -> p h c"),
                    in_=ctxt[:])
                if fqg == 3:
                    nc.gpsimd.collective_compute(
                        "AllToAll", mybir.AluOpType.bypass,
                        replica_groups=[list(range(NCORES))],
                        ins=[cc_in[fb][fhl].opt()],
                        outs=[cc_out[fb][fhl].opt()])

            def emit_attn_qg(b, hl, qg, filler=None):
                nkt = 4 * (qg + 1)    # causal 128-wide k-tiles
                npair = nkt // 2
                # E^T slab, flat [k-tile * 512 q] free layout
                et = p2.tile([128, 16 * 512], BF16, tag="et", bufs=3,
                             name="et")
                ctx_ps = pps.tile([128, 512], F32, tag="ctx", bufs=2,
                                  name="ctx")

                def off(kt):
                    r = kt - 4 * qg
                    return 128 * r if r > 0 else 0

                # causally-zero prefixes of the diagonal k-tiles
                for kt in range(4 * qg + 1, nkt):
                    nc.gpsimd.memset(et[:, ds(512 * kt, off(kt))], 0.0)

                def emit_scores(j):
                    # pair j: k-tiles 2j, 2j+1 -> one 2-bank tile
                    sc_ps = pps.tile([128, 1024], F32, tag="sc", bufs=3,
                                     name="sc")
                    for u in range(2):
                        kt = 2 * j + u
                        o = off(kt)
                        nc.tensor.matmul(
                            sc_ps[:, ds(512 * u + o, 512 - o)],
                            k_res[b][:, ts(kt, 128)],
                            q_res[hl][b][:, ds(qg * 512 + o, 512 - o)],
                            start=True, stop=True)
                    if 2 * j >= 4 * qg:
                        # diagonal pair: per-tile exp on the valid q-suffix +
                        # zero the 128-wide triangle
                        for u in range(2):
                            kt = 2 * j + u
                            o = off(kt)
                            nc.scalar.activation(
                                et[:, ds(512 * kt + o, 512 - o)],
                                sc_ps[:, ds(512 * u + o, 512 - o)], AF.Exp)
                            nc.gpsimd.affine_select(
                                out=et[:, ds(512 * kt + o, 128)],
                                in_=et[:, ds(512 * kt + o, 128)],
                                compare_op=mybir.AluOpType.is_ge,
                                fill=0.0, base=0, pattern=[[1, 128]],
                                channel_multiplier=-1)
                    else:
                        # off-diagonal pair: one fused exp
                        nc.scalar.activation(et[:, ds(512 * 2 * j, 1024)],
                                             sc_ps[:, :], AF.Exp)

                def emit_pv(j, is_last):
                    for u in range(2):
                        kt = 2 * j + u
                        o = off(kt) if kt > 0 else 0
                        nc.tensor.matmul(
                            ctx_ps[:, ds(o, 512 - o)],
                            v_res[b][:, kt, :],
                            et[:, ds(512 * kt + o, 512 - o)],
                            start=(kt == 0), stop=(is_last and u == 1),
                            skip_group_check=True)

                # pair order: kt0's pair first (it opens the ctx
                # accumulation), then diagonal pairs (their exp+select chain
                # is longest), then the rest
                diag = [p_ for p_ in range(npair) if 2 * p_ >= 4 * qg]
                order = ([p_ for p_ in (0,) if p_ not in diag] + diag
                         + [p_ for p_ in range(1, npair) if p_ not in diag])
                LOOK = 3
                for idx, p_ in enumerate(order):
                    emit_scores(p_)
                    if idx == 0 and pend[0] is not None:
                        finish()
                    if idx >= LOOK:
                        emit_pv(order[idx - LOOK],
                                order[idx - LOOK] == order[-1])
                # the interleaved projection chunk runs while the last
                # exps/selects of this q-group resolve
                if filler is not None:
                    filler()
                for idx in range(max(len(order) - LOOK, 0), len(order)):
                    emit_pv(order[idx], order[idx] == order[-1])

                # row sums: pairwise bf16 tree, diagonal subtree on GpSimd
                def tree(lo, hi, depth, eng, tagp):
                    if hi - lo == 1:
                        return et[:, ds(512 * lo, 512)]
                    mid = (lo + hi) // 2
                    a = tree(lo, mid, depth + 1, eng, tagp)
                    b_ = tree(mid, hi, depth + 1, eng, tagp)
                    t = p2s.tile([128, 512], BF16, tag=f"{tagp}{depth}",
                                 bufs=2, name="tr")
                    eng.tensor_add(t[:], a[:], b_[:])
                    return t

                if qg == 0:
                    padd16 = tree(0, 4, 0, nc.vector, "tr")
                else:
                    a = tree(0, 4 * qg, 1, nc.vector, "tr")
                    b_ = tree(4 * qg, nkt, 1, nc.gpsimd, "gtr")
                    padd16 = p2s.tile([128, 512], BF16, tag="tr0", bufs=2)
                    nc.vector.tensor_add(padd16[:], a[:], b_[:])
                pend[0] = (b, hl, qg, ctx_ps, padd16)

            # ---- emission schedule ----
            # prefix: both KV chains early (each exchange has ~45us latency)
            for sc in range(4):
                emit_chain("kv", 0, sc, xt_b0)
            # now request batch-1's kv slices (behind kv(b0)'s staging DMA)
            for sc4 in range(4):
                nc.sync.dma_start(out=xt_kv1[:, sc4, :],
                                  in_=xt.ap()[:, 4 + sc4, :])
            for sc in range(4):
                emit_chain("q0", 0, sc, xt_b0)
            for sc in range(4):
                emit_chain("q1", 0, sc, xt_b0)
            for sc in range(4):
                emit_chain("kv", 1, sc, xt_kv1)
            p1xB_cm.__exit__(None, None, None)
            p1xA_cm.__exit__(None, None, None)
            p2 = ctx.enter_context(tc.tile_pool(name="p2", bufs=2))
            p2s = ctx.enter_context(tc.tile_pool(name="p2s", bufs=4))
            p1xC_cm = tc.tile_pool(name="p1xC", bufs=1)
            p1xC = p1xC_cm.__enter__()
            xt_q1 = p1xC.tile([128, 4, 16 * TC], BF16, name="xt_q1")

            # attention schedule: batch-1 q-projection chunks interleaved one
            # per q-group (each filler runs while that q-group's last
            # exp/selects resolve); batch-0 h0 fillerless (its window re-
            # fetches batch-1's x^T slices, emitted at qg1 so they never
            # compete with the KV exchanges). q0(b1) chunk i must land before
            # attn(b1,h0) q-group i, q1(b1) chunk i before (b1,h1) qg i.
            fill = [None, None, None, None,
                    ("q0", 1, 0), ("q0", 1, 1), ("q0", 1, 2), ("q0", 1, 3),
                    ("q1", 1, 0), ("q1", 1, 1), ("q1", 1, 2), ("q1", 1, 3),
                    None, None, None, None]
            fi = 0
            wd_sb, g_all = None, None
            for ab, ahl in ((0, 0), (0, 1), (1, 0), (1, 1)):
                for qg in range(4):
                    if fi == 1:
                        # re-fetch batch-1 x^T slices (DMA is idle here)
                        for sc4 in range(4):
                            nc.sync.dma_start(out=xt_q1[:, sc4, :],
                                              in_=xt.ap()[:, 4 + sc4, :])
                    if fi == 12:
                        # batch-1 q chains done: free their x^T, start the
                        # dense weight load, allocate gather tiles
                        p1xC_cm.__exit__(None, None, None)
                        wdp = ctx.enter_context(tc.tile_pool(name="wdp",
                                                             bufs=1))
                        wd_sb = wdp.tile([128, 16, HIDDEN], BF16,
                                         name="wd_sb")
                        nc.sync.dma_start(out=wd_sb[:], in_=wd.ap())
                        p3g = ctx.enter_context(tc.tile_pool(name="p3g",
                                                             bufs=1))
                        g_all = [[p3g.tile([128, NCORES, 256], BF16,
                                           name=f"g{b}{h}")
                                  for h in range(2)] for b in range(B)]
                    args = fill[fi]
                    fi += 1
                    emit_attn_qg(ab, ahl, qg,
                                 filler=(None if args is None else
                                         (lambda a=args:
                                          emit_chain(*a, xt_q1))))
            finish()
            for b in range(B):
                for hl in range(2):
                    nc.sync.dma_start(
                        out=g_all[b][hl][:],
                        in_=cc_out[b][hl].rearrange("i p s -> p i s"))

            # ---- dense projection on my 256-token slice per batch ----
            # accumulators borrow two score-ring slots (2 banks each)
            with tc.tile_pool(name="p3", bufs=2) as p3:
                for b in range(B):
                    for u in range(2):
                        o_ps = [pps.tile([128, 1024], F32, tag="sc", bufs=3,
                                         name=f"ops{h_}") for h_ in range(2)]
                        o_sb = p3.tile([128, HIDDEN], F32, tag="osb", bufs=1)
                        # hl-major so the first half only needs g_all[b][0]
                        for ec in range(16):
                            hl, i = ec // 8, ec % 8
                            for oc in range(4):
                                nc.tensor.matmul(
                                    o_ps[oc // 2][:, ts(oc % 2, 512)],
                                    g_all[b][hl][:, i, ts(u, 128)],
                                    wd_sb[:, 2 * i + hl, ts(oc, 512)],
                                    start=(ec == 0), stop=(ec == 15),
                                    skip_group_check=True)
                        nc.scalar.copy(o_sb[:, ts(0, 1024)], o_ps[0][:])
                        nc.vector.tensor_copy(o_sb[:, ts(1, 1024)],
                                              o_ps[1][:])
                        nc.sync.dma_start(
                            out=out.ap()[ds(b * 256 + u * 128, 128), :],
                            in_=o_sb[:])

    nc.compile()
    return nc


def kernel(x, w_q, w_kv, w_dense):
    from concourse.bass_utils import run_bass_kernel_spmd

    bf16 = ml_dtypes.bfloat16
    x = np.asarray(x, dtype=np.float32)
    w_q = np.asarray(w_q, dtype=np.float32)
    w_kv = np.asarray(w_kv, dtype=np.float32)
    w_dense = np.asarray(w_dense, dtype=np.float32)

    # x^T pre-tiled to [p, tchunk, hc, t'] so one DMA trigger moves a
    # 512-token slice of every hidden chunk as one 16KB run per partition
    xt = np.ascontiguousarray(
        x.reshape(T, HIDDEN).T.reshape(16, 128, NTC, TC).transpose(1, 2, 0, 3)
        .reshape(128, NTC, 16 * TC)
    ).astype(bf16)
    wq_s = (w_q * SCALE).astype(bf16)          # fold softmax scale into Q proj
    wkv_b = w_kv.astype(bf16)
    wd_b = w_dense.astype(bf16)

    def pretile(w):
        # [2048, e] -> SBUF layout [p, hc*e]: row p, col hc*e_sz + e
        e_sz = w.shape[1]
        return np.ascontiguousarray(
            w.reshape(16, 128, e_sz).transpose(1, 0, 2).reshape(128, 16 * e_sz))

    wd_t = pretile(wd_b)
    in_maps = []
    for c in range(NCORES):
        g = c // 2
        if c % 2 == 0:
            wkv_c = wkv_b[:, 128 * g:128 * (g + 1)]                # K half
        else:
            wkv_c = wkv_b[:, 512 + 128 * g:512 + 128 * (g + 1)]    # V half
        in_maps.append({
            "xt": xt,
            "wq": pretile(wq_s[:, 256 * c:256 * (c + 1)]),
            "wkv": pretile(wkv_c),
            "wd": wd_t,
        })

    if "nc" not in _CACHE:
        _CACHE["nc"] = _build()
    nc = _CACHE["nc"]

    res = run_bass_kernel_spmd(nc, in_maps, core_ids=list(range(NCORES)))
    kernel.last_results = res
    kernel.last_exec_time_ns = res.exec_time_ns

    out_full = np.empty((T, HIDDEN), dtype=np.float32)
    for c in range(NCORES):
        r = res.results[c]["out"]              # [512, 2048]
        for b in range(B):
            out_full[b * SQ + 256 * c: b * SQ + 256 * (c + 1), :] = \
                r[b * 256:(b + 1) * 256, :]
    return out_full.reshape(B, SQ, HIDDEN)
